# revision 6
# baseline (speedup 1.0000x reference)
"""CosFace loss (B=1024, D=512, C=100000) on 8 Trainium2 NeuronCores.

Strategy (tensor-parallel classification head over classes):
  - Classes sharded 12500/core (padded to 12544 = 98*128 zero rows).
  - Host prep: rows of x and w L2-normalized, scaled by alpha=sqrt(S*log2e)
    and quantized to fp8 e4m3 (TRN format == ml_dtypes.float8_e4m3 for
    |v|<=240).  The fp8 DoubleRow matmul then produces PSUM values
    p = (S*log2e)*cos -- logits in the log2 domain -- at 0.5 cycles/row
    (4x the fp32r rate), contracting 2 k-tiles (256 elems) per instr.
  - exp+sum: fixed shift (no running max needed: args in [-128, 0]):
      ACT path: exp(ln2*p - 64) with accum_out per (m, chunk).
      DVE path (EXP_SPLIT>0): custom DVE op pair registered at import:
        pass1  v = cubic(p) ~ 2^((p - S*log2e)/128)   (4 consts, C3 spill)
        pass2  v^128 by 7 squarings, accum_out=partial sum
      which makes the Vector engine a second exp engine at ~half ACT rate.
  - One 4KB AllReduce of the [128, 8] per-row partial sums.
  - Margin fixup on host: the label logit must be S*(cos-M), not S*cos.
    Host computes cos_label exactly (fp64) and ships dneg[b] =
    exp(S cos_l - 64) - exp(S cos_l - S M - 64) plus t[b] = S cos_l; the
    device computes loss = mean(64 + ln(sum - dneg) + S*M - t), with the
    ACT Ln prescaled by 2^66 (folded into the final bias) for accuracy.
"""

import math
import os as _os

import numpy as np
import ml_dtypes

import concourse.bass as bass
import concourse.mybir as mybir
import concourse.tile as tile
from concourse import bacc
from concourse.bass_utils import run_bass_kernel_spmd

B, D, C = 1024, 512, 100000
S, MARGIN = 64.0, 0.35
LOG2E = 1.4426950408889634
LN2 = 0.6931471805599453
ALPHA = math.sqrt(S * LOG2E)   # quant scale; alpha^2 = S*log2e
SHIFT = 64.0
NCORES = 8
CSHARD = C // NCORES          # 12500 real classes per core
CLOC = 12544                  # padded (98 * 128)
KT = D // 128                 # 4 k-tiles of 128 (DoubleRow eats 2 at a time)
MT = B // 128                 # 8 batch tiles
CHUNK = int(_os.environ.get('KCHUNK', '2048'))   # classes per PSUM tile
NCHUNK = (CLOC + CHUNK - 1) // CHUNK
PBUFS = int(_os.environ.get('PBUFS', '2'))
# how many full chunks per m go to the DVE exp path (0 = ACT only).
# Measured on HW (R8-R1 steady state): EXP_SPLIT=0 -> 91.0us (ACT-bound),
# EXP_SPLIT=1 -> 74.9us (balanced), EXP_SPLIT=4/KCHUNK=1024 -> 115us
# (custom-DVE ops measure ~1.8ns/elem, ~1.7x the generic DVE model).
EXP_SPLIT = int(_os.environ.get('EXP_SPLIT', '1'))

F32 = mybir.dt.float32
F8 = mybir.dt.float8e4
AF = mybir.ActivationFunctionType
AX = mybir.AxisListType
ALU = mybir.AluOpType
DR = mybir.MatmulPerfMode.DoubleRow

_NC = None
LAST_RESULTS = None

# ---- custom DVE exp2 ops (registered once, at import) ----------------------
# cubic coefficients for Q(p) ~ 2^((p - S*log2e)/128) on p in [-92.35, 92.35],
# relative-error weighted; elements with p - 92.33 < -45 (true value < 2^-45
# relative to the per-row sum) are allowed ~0.3% pre-amp error -- after ^128
# they underflow fp32 to ~0 anyway.  Fit at import (deterministic, ~ms).
_EXP2_OPS = None


def _fit_poly():
    pg = np.linspace(-92.36, 92.36, 20001)
    yt = np.exp2((pg - S * LOG2E) / 128.0)
    wgt = np.where(pg - S * LOG2E >= -45.0, 1.0 / yt, 0.02 / yt)
    V = np.vander(pg, 4)
    cf, *_ = np.linalg.lstsq(V * wgt[:, None], yt * wgt, rcond=None)
    # tune the constant term so the post-amplification mean error ~ 0 on the
    # contributing zone (reduces the systematic bias of the cubic)
    zone = pg - S * LOG2E >= -45.0
    amp = ((np.vander(pg, 4) @ cf) / yt)[zone] ** 128
    cf = cf.copy()
    cf[3] -= yt[zone].mean() * 0  # placeholder (mean already ~1 from the fit)
    bias = amp.mean() ** (1.0 / 128.0)
    cf /= bias
    return [float(c) for c in cf]


def _register_exp2_ops():
    global _EXP2_OPS
    if _EXP2_OPS is not None:
        return _EXP2_OPS
    from concourse import dve_ops
    from concourse.dve_spec import (C0, C1, C2, C3, Spec, Src0, Zero, lower,
                                    _spill_c3_to_src1, sq)
    from concourse.dve_uop import DveOpSpec
    from operator import add

    cf = _fit_poly()

    def _mk(name, spec):
        # compute the uops sha so the drift guard passes, then register the
        # op in the process-local tables (the documented extension point is
        # appending to dve_ops.OPS; rows [1, 0x20) are free, 16 used).
        probe = DveOpSpec(name=name, opcode=0, uops=lower(spec, ver="v3"),
                          rd1_en=False)
        op = dve_ops.DveOp(name, spec, subdim=False,
                           uops_sha={"v3": probe.sha("v3")})
        if name not in dve_ops._SUB_OPCODE_FOR_NAME:
            row = max(dve_ops._SUB_OPCODE_FOR_NAME.values()) + 1
            assert row < 0x20
            dve_ops._SUB_OPCODE_FOR_NAME[name] = row
            dve_ops.OPS.append(op)
            dve_ops.CUSTOM_DVE_SPECS[name] = op.spec
        return op

    # pass 1: v = ((c0*p + c1)*p + c2)*p + c3   (c3 spilled to Src1 [P,1])
    def _p1_ref(in0, in1, s0, s1, imm2):
        x = in0.astype(np.float32)
        c3 = np.asarray(in1, np.float32).reshape(-1, 1)
        return (((np.float32(s0) * x + np.float32(s1)) * x
                 + np.float32(imm2)) * x + c3).astype(np.float32)

    body1 = _spill_c3_to_src1(((Src0 * C0 + C1) * Src0 + C2) * Src0 + C3)
    op1 = _mk("EXP2CUBE_ANT", Spec(body=body1, reference=_p1_ref))

    # pass 2: out = v^128 (7 squarings); accum_out = sum(out)
    def _p2_ref(in0, in1, s0, s1, imm2):
        v = in0.astype(np.float32)
        for _ in range(7):
            v = (v * v).astype(np.float32)
        return v, v.reshape(v.shape[0], -1).sum(axis=-1, keepdims=True).astype(np.float32)

    b2 = Src0
    for _ in range(7):
        b2 = sq(b2)
    op2 = _mk("EXP2POW_ANT", Spec(body=b2, accum=add, accum_init=Zero,
                                  reference=_p2_ref))
    _EXP2_OPS = (op1, op2, cf)
    return _EXP2_OPS


def _body(nc, tc, xq, wq, tin, dneg, loss, collective=True):
    from contextlib import ExitStack
    with ExitStack() as ctx:
        singles = ctx.enter_context(tc.tile_pool(name="singles", bufs=1))
        ins = ctx.enter_context(tc.tile_pool(name="ins", bufs=2))
        wpool = ctx.enter_context(tc.tile_pool(name="wpool", bufs=2))
        psump = ctx.enter_context(tc.tile_pool(name="psump", bufs=PBUFS, space="PSUM"))
        scrp = ctx.enter_context(tc.tile_pool(name="scrp", bufs=3))
        dram = ctx.enter_context(tc.tile_pool(name="dram", bufs=2, space="DRAM"))

        # ---- resident inputs ----
        wq_sb = wpool.tile([128, KT, CLOC], F8, tag="wq")
        nc.sync.dma_start(out=wq_sb[:, :, :],
                          in_=wq.ap().rearrange("(k p) c -> p k c", p=128))
        xq_sb = ins.tile([128, KT, B], F8, tag="xq")
        nc.sync.dma_start(out=xq_sb[:, :, :],
                          in_=xq.ap().rearrange("(k p) b -> p k b", p=128))
        t_sb = ins.tile([128, MT], F32, tag="t")
        nc.sync.dma_start(out=t_sb[:, :], in_=tin.ap())
        dn_sb = ins.tile([128, MT], F32, tag="dn")
        nc.sync.dma_start(out=dn_sb[:, :], in_=dneg.ap())

        def const_col(val, tag):
            t = singles.tile([128, 1], F32, tag=tag)
            nc.vector.memset(t[:, :], val)
            return t

        cb_m64 = const_col(-SHIFT, "cb_m64")
        cb_fin = const_col(SHIFT + S * MARGIN - 66.0 * LN2, "cb_fin")

        use_dve = EXP_SPLIT > 0
        if use_dve:
            op1, op2, cf = _register_exp2_ops()
            c3_col = const_col(cf[3], "c3_col")

        # ---- main loop: fp8 DoubleRow matmuls + exp/accumulate ----
        # partials[:, m, j]: per-(m, chunk) partial sums of exp
        parts = singles.tile([128, MT, NCHUNK], F32)
        wv = wq_sb
        for m in range(MT):
            for c in range(NCHUNK):
                c0 = c * CHUNK
                ncls = min(CHUNK, CLOC - c0)
                g = psump.tile([128, CHUNK], F32, tag="g")
                for k in range(KT // 2):
                    lhsT = xq_sb[:, 2 * k:2 * k + 2, m * 128:(m + 1) * 128]
                    for n in range(0, ncls, 512):
                        nsz = min(512, ncls - n)
                        nc.tensor.matmul(g[:, n:n + nsz], lhsT,
                                         wv[:, 2 * k:2 * k + 2, c0 + n:c0 + n + nsz],
                                         start=(k == 0), stop=(k == KT // 2 - 1),
                                         perf_mode=DR)
                # chunks [0, EXP_SPLIT) and the short tail chunk of each m
                # go to the DVE exp path (tail on DVE rebalances ACT, the
                # measured limiter at EXP_SPLIT=1)
                if use_dve and (c < EXP_SPLIT or ncls < CHUNK):
                    v = scrp.tile([128, CHUNK], F32, tag="v")
                    nc.vector._custom_dve(op1, out=v[:, :ncls], in0=g[:, :ncls],
                                          in1=c3_col[:, :], s0=cf[0], s1=cf[1],
                                          imm2=cf[2])
                    nc.vector._custom_dve(op2, out=v[:, :ncls], in0=v[:, :ncls],
                                          accum_out=parts[:, m, c:c + 1])
                else:
                    # in-place on PSUM; free affine maps p -> ln2*p - 64
                    nc.scalar.activation(g[:, :ncls], g[:, :ncls], AF.Exp,
                                         bias=cb_m64[:, :], scale=LN2,
                                         accum_out=parts[:, m, c:c + 1])

        # ---- reduce partials + AllReduce ----
        se_part = singles.tile([128, MT], F32)
        nc.vector.tensor_reduce(se_part[:, :], parts[:, :, :], axis=AX.X, op=ALU.add)
        full_se = singles.tile([128, MT], F32)
        if collective:
            ar_in = dram.tile([128, MT], F32, tag="ari")
            ar_out = dram.tile([128, MT], F32, tag="aro", addr_space="Shared")
            nc.sync.dma_start(out=ar_in[:, :], in_=se_part[:, :])
            nc.gpsimd.collective_compute(
                "AllReduce", ALU.add,
                replica_groups=[list(range(NCORES))],
                ins=[ar_in.opt()], outs=[ar_out.opt()])
            nc.sync.dma_start(out=full_se[:, :], in_=ar_out[:, :])
        else:
            nc.vector.tensor_scalar_mul(full_se[:, :], se_part[:, :], float(NCORES))

        # ---- logz and loss ----
        adj = singles.tile([128, MT], F32)
        nc.vector.tensor_sub(adj[:, :], full_se[:, :], dn_sb[:, :])
        ln_adj = singles.tile([128, MT], F32)
        # ACT Ln is inaccurate for tiny args; prescale by 2^66 (constant
        # folded into the final bias: ln(adj*2^66) = ln(adj) + 66*ln2)
        nc.scalar.activation(ln_adj[:, :], adj[:, :], AF.Ln, scale=float(2.0 ** 66))
        lossv = singles.tile([128, MT], F32)
        nc.vector.tensor_sub(lossv[:, :], ln_adj[:, :], t_sb[:, :])
        rowsum = singles.tile([128, 1], F32)
        junk2 = singles.tile([128, MT], F32)
        nc.scalar.activation(junk2[:, :], lossv[:, :], AF.Identity,
                             accum_out=rowsum[:, :])
        # partition-axis reduce on PE: [1,1] = ones.T @ rowsum
        ones_col = singles.tile([128, 1], F32)
        nc.vector.memset(ones_col[:, :], 1.0)
        fin_ps = psump.tile([1, 1], F32, tag="g")
        nc.tensor.matmul(fin_ps[:, :], ones_col[:, :], rowsum[:, :],
                         start=True, stop=True)
        fin = singles.tile([1, 1], F32)
        nc.scalar.activation(fin[:, :], fin_ps[:, :], AF.Identity,
                             scale=1.0 / B, bias=cb_fin[:1, :])
        nc.sync.dma_start(out=loss.ap()[:, :], in_=fin[:, :])


def _build(repeat=1, collective=True):
    nc = bacc.Bacc("TRN2", target_bir_lowering=False, debug=False,
                   num_devices=NCORES)
    xq = nc.dram_tensor("xq", [D, B], F8, kind="ExternalInput")
    wq = nc.dram_tensor("wq", [D, CLOC], F8, kind="ExternalInput")
    tin = nc.dram_tensor("tin", [128, MT], F32, kind="ExternalInput")
    dneg = nc.dram_tensor("dneg", [128, MT], F32, kind="ExternalInput")
    loss = nc.dram_tensor("loss", [1, 1], F32, kind="ExternalOutput")
    with tile.TileContext(nc) as tc:
        for _ in range(repeat):
            _body(nc, tc, xq, wq, tin, dneg, loss, collective=collective)
    nc.compile()
    return nc


def _get_nc():
    global _NC
    if _NC is None:
        _NC = _build()
    return _NC


def _prep(inputs):
    f8 = ml_dtypes.float8_e4m3
    x = np.asarray(inputs["input"], dtype=np.float32)
    label = np.asarray(inputs["label"]).astype(np.int64)
    w = np.asarray(inputs["weight"], dtype=np.float32)
    xn = x / np.maximum(np.sqrt((x * x).sum(axis=1, keepdims=True)), 1e-12)
    wnorm = np.sqrt((w * w).sum(axis=1, keepdims=True, dtype=np.float32))
    wn = w / np.maximum(wnorm, 1e-12)
    xq = np.ascontiguousarray((xn * ALPHA).T).astype(f8)
    # margin fixup constants from the exact (fp64) label cosines
    cosl = np.einsum('bd,bd->b', xn.astype(np.float64), wn[label].astype(np.float64))
    tin = (S * cosl).astype(np.float32).reshape(MT, 128).T.copy()
    dneg64 = np.exp(S * cosl - SHIFT) - np.exp(S * cosl - S * MARGIN - SHIFT)
    dneg = dneg64.astype(np.float32).reshape(MT, 128).T.copy()
    wnT = np.ascontiguousarray((wn * ALPHA).T.astype(f8))  # [D, C] fp8
    in_maps = []
    for k in range(NCORES):
        shard = np.zeros((D, CLOC), dtype=f8)
        shard[:, :CSHARD] = wnT[:, k * CSHARD:(k + 1) * CSHARD]
        in_maps.append({"xq": xq, "wq": shard, "tin": tin, "dneg": dneg})
    return in_maps


def kernel(**inputs):
    global LAST_RESULTS
    _os.environ["BASS_NEVER_TRACE"] = "1"
    nc = _get_nc()
    in_maps = _prep(inputs)
    res = run_bass_kernel_spmd(nc, in_maps, core_ids=list(range(NCORES)))
    LAST_RESULTS = res
    return np.asarray(res.results[0]["loss"][0, 0], dtype=np.float32)
